# revision 17
# baseline (speedup 1.0000x reference)
"""Trainium2 Bass kernel for nn_Attention_51548197487430.

Multi-head attention (B=2, S=2048, D=1024, H=16, HD=64), fp32 reference,
sharded 2 heads per core across 8 NeuronCores (head/tensor parallel per
the sharding hint: w_qkv output dim and w_out input dim split per-head;
the all-reduce after the output projection is realized as the host-side
unshard step, which sums the 8 partial outputs).

Per-core device kernel (SPMD, identical program; per-core weight slices):
  inputs (host pre-laid-out, cast to bf16):
    xT     (1024, 4096)  x^T, both batches side by side   [same on all cores]
    wqkvT  (1024, 384)   [Wq_h0|Wq_h1|Wk_h0|Wk_h1|Wv_h0|Wv_h1]^T for this core
    woutT  (128, 1024)   w_out[:, 128c:128c+128]^T
  output:
    out    (4096, 1024)  fp16 partial x_out (pre-bias); host upcasts and
                         sums the 8 partials in fp32, then adds b_out

Fully software-pipelined schedule (v2):
  - Only QKV s-chunk 0 is projected serially; s1..s7 projections are
    spread through the attention k-loops as PE filler so the scalar
    engine (exp) and PE run concurrently from ~6us in.
  - Attention inner loop is pipelined 2-deep: scores(k+1) issues before
    attnV(k-1), so the PE never waits on the exp of the current chunk.
  - V^T -> V transposes go through the DMA xbar (dma_start_transpose)
    instead of the PE+DVE path.
  - Softmax reciprocal is computed on the scalar engine as exp(-ln d)
    (both functions live in the natural_log_exp_and_others table set),
    freeing the DVE of the very slow RECIPROCAL op.
  - On the last three (ACT-bound) iterations, 4 of 16 exp chunks are
    computed on the DVE with a Schraudolph bit-trick (int16 bf16-bit
    arithmetic, ~3% max weight error, cancels in softmax), keeping every
    engine below the PE's per-iteration work.
  - The final iteration's normalize/output-projection tail is emitted
    fine-grained so it pipelines across engines instead of serializing.
"""

import numpy as np

B, S, D, H, HD = 2, 2048, 1024, 16, 64
N_CORES = 8
SCALE = HD ** (-0.5)
BS = B * S               # 4096
SC = 512                 # qkv-phase s-chunk (8 chunks)
QC = 512                 # attention q-chunk
NKC = S // 128           # 16 k-chunks per batch
DC = D // 128            # 8 contraction chunks

# Schraudolph exp in bf16-bit space: exp(s*SCALE) ~= bitcast_bf16(int16(
#   s * SCALE*128/ln2 + (16256 - 5.51 + 0.5) ))
EXP_MUL = SCALE * 128.0 / float(np.log(2.0))
EXP_BIAS = 16251.0

_cache = {}

import os
K_NO_DMAT = bool(int(os.environ.get("K_NO_DMAT", "0")))  # PE transpose path
K_NO_I16 = bool(int(os.environ.get("K_NO_I16", "0")))    # all exp on ACT
K_NO_LN = bool(int(os.environ.get("K_NO_LN", "0")))      # DVE reciprocal
K_NOFILL = bool(int(os.environ.get("K_NOFILL", "0")))    # no k-loop fillers
K_DEPTH = int(os.environ.get("K_DEPTH", "2"))            # attnV lag chunks


def _build():
    import concourse.bass as bass
    import concourse.mybir as mybir
    import concourse.tile as tile
    from concourse import bacc

    F32 = mybir.dt.float32
    F32R = mybir.dt.float32r
    BF16 = mybir.dt.bfloat16
    F16 = mybir.dt.float16
    I16 = mybir.dt.int16
    AF = mybir.ActivationFunctionType
    ALU = mybir.AluOpType

    nc = bacc.Bacc("TRN2", target_bir_lowering=False, debug=False,
                   num_devices=N_CORES)
    xT_d = nc.dram_tensor("xT", (D, BS), BF16, kind="ExternalInput").ap()
    wqkvT_d = nc.dram_tensor("wqkvT", (D, 384), BF16, kind="ExternalInput").ap()
    woutT_d = nc.dram_tensor("woutT", (128, D), BF16, kind="ExternalInput").ap()
    # fp16 partials: |values| << 1, so fp16's 10-bit mantissa beats bf16
    # and halves the output DMA; host upcasts and sums in fp32.
    out_d = nc.dram_tensor("out", (BS, D), F16, kind="ExternalOutput").ap()

    with tile.TileContext(nc) as tc:
        with tc.tile_pool(name="persist", bufs=1) as persist, \
             tc.tile_pool(name="xin", bufs=4) as xin, \
             tc.tile_pool(name="epool", bufs=5) as epool, \
             tc.tile_pool(name="pod", bufs=4) as pod, \
             tc.tile_pool(name="pinv", bufs=4) as pinv, \
             tc.tile_pool(name="pot", bufs=2) as pot, \
             tc.tile_pool(name="posb", bufs=3) as posb, \
             tc.tile_pool(name="ps_sc", bufs=2, space="PSUM") as ps_sc, \
             tc.tile_pool(name="pacc", bufs=2, space="PSUM") as pacc, \
             tc.tile_pool(name="ptr", bufs=2, space="PSUM") as ptr:

            # ---- input DMAs first so transfers start immediately ----
            xts = {}

            def emit_xt_dma(s):
                xt = xin.tile([128, DC, SC], BF16, tag="xt", name="xt")
                nc.sync.dma_start(
                    xt[:], xT_d[:, s * SC:(s + 1) * SC]
                    .rearrange("(po pi) s -> pi po s", pi=128))
                xts[s] = xt

            emit_xt_dma(0)
            wqkvT = persist.tile([128, DC, 384], BF16, tag="wqkvT")
            nc.sync.dma_start(wqkvT[:], wqkvT_d.rearrange(
                "(po pi) e -> pi po e", pi=128))
            emit_xt_dma(1)
            woutT = persist.tile([128, D], BF16, tag="woutT")
            nc.sync.dma_start(woutT[:], woutT_d)
            emit_xt_dma(2)

            # ---- persistent tiles ----
            QT = persist.tile([128, BS], BF16, tag="QT")
            KT = persist.tile([128, BS], BF16, tag="KT")
            VT = persist.tile([128, BS], F32 if K_NO_DMAT else BF16,
                              tag="VT")
            # V_aug[b][h]: (128, NKC, 128); h0 = [V | ones], h1 = [ones | V]
            vaug = [[persist.tile([128, NKC, 128], BF16, tag=f"vaug{b}{h}",
                                  name=f"vaug{b}{h}")
                     for h in range(2)] for b in range(B)]
            # constants: memset supports fp32 only; DVE copy rounds/casts
            const_f32 = persist.tile([128, NKC * 64], F32, tag="const_f32")
            nc.gpsimd.memset(const_f32[:], 1.0)
            # inv2: anti-block-diagonal 1/64 weights; one matmul pair
            # accumulates both heads' denominator broadcasts into ONE psum
            # tile (h0's denom -> partitions 0-63, h1's -> 64-127).
            inv2 = persist.tile([128, 128], F32R, tag="inv2")
            inv2_f32 = persist.tile([128, 128], F32, tag="inv2_f32")
            nc.gpsimd.memset(inv2_f32[:], 0.0)
            nc.gpsimd.memset(inv2_f32[64:128, 0:64], 1.0 / 64.0)
            nc.gpsimd.memset(inv2_f32[0:64, 64:128], 1.0 / 64.0)
            nc.vector.tensor_copy(inv2[:], inv2_f32[:])
            ones_3d = const_f32[:].rearrange("p (a b) -> p a b", b=64)
            for b in range(B):
                nc.vector.tensor_copy(vaug[b][0][:, :, 64:128], ones_3d)
                nc.vector.tensor_copy(vaug[b][1][:, :, 0:64], ones_3d)

            # ---- qkv projection pieces (as schedulable closures) ----
            def mk_qkv_mms(s, e, d0, d1, state):
                """Matmuls d0..d1-1 of e-chunk e (0=V,1=K,2=Q by caller's
                ordering) for s-chunk s; allocates psum on d0==0, casts to
                the destination on d1==DC."""
                def fn():
                    if d0 == 0:
                        state["ps"] = ptr.tile([128, SC], F32, tag="ptr",
                                               name="qkv_ps")
                    for d in range(d0, d1):
                        nc.tensor.matmul(
                            state["ps"][:],
                            lhsT=wqkvT[:, d, 128 * e:128 * (e + 1)],
                            rhs=xts[s][:, d, :],
                            start=(d == 0), stop=(d == DC - 1))
                    if d1 == DC:
                        dst = (QT, KT, VT)[e]
                        nc.vector.tensor_copy(
                            dst[:, s * SC:(s + 1) * SC], state["ps"][:])
                return fn

            if K_NO_DMAT:
                ident = persist.tile([128, 128], F32, tag="ident")
                from concourse.masks import make_identity
                make_identity(nc, ident[:])

            def mk_vtrans(j):
                """DMA-xbar transpose of VT cols [j*128,(j+1)*128) into the
                V_aug tiles (head h's 64 v-dims -> its value columns)."""
                def fn():
                    b, k = divmod(j, NKC)
                    sl = slice(j * 128, (j + 1) * 128)
                    if K_NO_DMAT:
                        ps = ptr.tile([128, SC], F32, tag="ptr", name="vt_ps")
                        pt = ps[:, 0:128]
                        nc.tensor.transpose(pt, VT[:, sl], ident[:])
                        nc.vector.tensor_copy(vaug[b][0][:, k, 0:64],
                                              pt[:, 0:64])
                        nc.vector.tensor_copy(vaug[b][1][:, k, 64:128],
                                              pt[:, 64:128])
                    else:
                        nc.sync.dma_start_transpose(
                            vaug[b][0][:, k, 0:64], VT[0:64, sl])
                        nc.sync.dma_start_transpose(
                            vaug[b][1][:, k, 64:128], VT[64:128, sl])
                return fn

            def qkv_closures(s, dma_s=None):
                """Closure list projecting s-chunk s (V,K first: V feeds the
                vtrans chain and K the next k-chunks' scores; Q is only
                needed one iteration later), plus optional xt prefetch."""
                cls = []
                if dma_s is not None:
                    cls.append(lambda: emit_xt_dma(dma_s))
                for e, name in ((2, "v"), (1, "k"), (0, "q")):
                    st = {}
                    cls.append(mk_qkv_mms(s, e, 0, 3, st))
                    cls.append(mk_qkv_mms(s, e, 3, 6, st))
                    cls.append(mk_qkv_mms(s, e, 6, 8, st))
                    if e == 2:
                        for j in range(4 * s, 4 * s + 4):
                            cls.append(mk_vtrans(j))
                return cls

            # ---- finish phase of one attention iteration ----
            def finish_closures(st):
                """Normalize + output projection for a completed iteration,
                as 11 closures to spread through the next k-loop."""
                cls = []

                def f_inv2():
                    st["pbc"] = ptr.tile([128, SC], F32, tag="ptr",
                                         name="pbc")
                    nc.tensor.matmul(st["pbc"][:], lhsT=inv2[64:128, :],
                                     rhs=st["odA"][64:128, :],
                                     start=True, stop=False)
                    nc.tensor.matmul(st["pbc"][:], lhsT=inv2[0:64, :],
                                     rhs=st["odB"][0:64, :],
                                     start=False, stop=True)

                def f_ln():
                    if K_NO_LN:
                        return
                    st["lnd"] = pinv.tile([128, QC], F32, tag="lnd",
                                          name="lnd")
                    nc.scalar.activation(st["lnd"][:], st["pbc"][:], AF.Ln)

                def f_expneg():
                    st["invd"] = pinv.tile([128, QC], F32, tag="invd",
                                           name="invd")
                    if K_NO_LN:
                        nc.vector.reciprocal(st["invd"][:], st["pbc"][:])
                    else:
                        nc.scalar.activation(st["invd"][:], st["lnd"][:],
                                             AF.Exp, scale=-1.0)

                def f_mul(h):
                    def fn():
                        if h == 0:
                            st["ot"] = pot.tile([128, QC], BF16, tag="ot",
                                                name="ot")
                            nc.vector.tensor_mul(
                                out=st["ot"][0:64, :],
                                in0=st["odA"][0:64, :].bitcast(F32),
                                in1=st["invd"][0:64, :])
                        else:
                            nc.vector.tensor_mul(
                                out=st["ot"][64:128, :],
                                in0=st["odB"][64:128, :].bitcast(F32),
                                in1=st["invd"][64:128, :])
                    return fn

                def f_outproj(j, e):
                    def fn():
                        if e == 0:
                            st[f"osb{j}"] = posb.tile([128, D], F16,
                                                      tag="osb", name="osb")
                        po = ptr.tile([128, SC], F32, tag="ptr", name="po")
                        nc.tensor.matmul(
                            po[:], lhsT=st["ot"][:, j * 128:(j + 1) * 128],
                            rhs=woutT[:, e * SC:(e + 1) * SC],
                            start=True, stop=True)
                        nc.vector.tensor_copy(
                            st[f"osb{j}"][:, e * SC:(e + 1) * SC], po[:])
                        if e == 1:
                            row = st["q0"] + j * 128
                            nc.sync.dma_start(out_d[row:row + 128, :],
                                              st[f"osb{j}"][:])
                    return fn

                cls.append(f_inv2)
                cls.append(f_ln)
                cls.append(f_expneg)
                cls.append(f_mul(0))
                cls.append(f_mul(1))
                for j in range(4):
                    cls.append(f_outproj(j, 0))
                    cls.append(f_outproj(j, 1))
                return cls

            # ---- one attention iteration, pipelined 2 deep ----
            def emit_attn(b, q, kmap, dve_ks=()):
                """softmax(QK^T*scale)V for (batch b, q-chunk q).  kmap maps
                k -> filler closures.  dve_ks: k-chunks whose exp runs on
                the DVE via the Schraudolph bit-trick.  attnV lags scores
                by 2 chunks so the PE never waits on the current exp."""
                q0 = b * S + q * QC
                psA = pacc.tile([128, SC], F32, tag="pacc", name="psA")
                psB = pacc.tile([128, SC], F32, tag="pacc", name="psB")
                ebs = {}
                for k in range(NKC + K_DEPTH):
                    if k < NKC:
                        kcol = b * S + k * 128
                        pss = ps_sc.tile([128, 2 * QC], F32, tag="scores")
                        nc.tensor.matmul(
                            pss[:, 0:QC], lhsT=KT[0:64, kcol:kcol + 128],
                            rhs=QT[0:64, q0:q0 + QC], start=True, stop=True)
                        nc.tensor.matmul(
                            pss[:, QC:2 * QC],
                            lhsT=KT[64:128, kcol:kcol + 128],
                            rhs=QT[64:128, q0:q0 + QC],
                            start=True, stop=True)
                        if k in dve_ks:
                            ebi = epool.tile([128, 2 * QC], I16, tag="eb",
                                             name="ebi")
                            nc.vector.tensor_scalar(
                                ebi[:], pss[:], EXP_MUL, EXP_BIAS,
                                ALU.mult, ALU.add)
                            ebs[k] = (ebi, True)
                        else:
                            eb = epool.tile([128, 2 * QC], BF16, tag="eb",
                                            name="eb")
                            nc.scalar.activation(eb[:], pss[:], AF.Exp,
                                                 scale=float(SCALE))
                            ebs[k] = (eb, False)
                    for fn in kmap.get(k, ()):
                        fn()
                    ka = k - K_DEPTH
                    if ka >= 0:
                        ebt, is_i16 = ebs.pop(ka)

                        def ebsl(lo, hi):
                            sl = ebt[:, lo:hi]
                            return sl.bitcast(BF16) if is_i16 else sl
                        nc.tensor.matmul(psA[:], lhsT=vaug[b][0][:, ka, :],
                                         rhs=ebsl(0, QC),
                                         start=(ka == 0),
                                         stop=(ka == NKC - 1))
                        nc.tensor.matmul(psB[:], lhsT=vaug[b][1][:, ka, :],
                                         rhs=ebsl(QC, 2 * QC),
                                         start=(ka == 0),
                                         stop=(ka == NKC - 1))
                # drain each accumulator with ONE full-tile copy so psA/psB
                # release ASAP (the next iteration's attn@V needs the slots).
                # odA = [o^T_h0 (0:64) | denom_h0 (64:128)], odB the mirror.
                odA = pod.tile([128, QC], F32R, tag="odA", name="odA")
                odB = pod.tile([128, QC], F32R, tag="odB", name="odB")
                nc.vector.tensor_copy(odA[:], psA[:])
                nc.vector.tensor_copy(odB[:], psB[:])
                return {"q0": q0, "odA": odA, "odB": odB}

            # ---- schedule assembly ----
            def spread(cls, ks):
                """Distribute closures over k-slots in contiguous blocks so
                relative emission order is preserved (accumulation groups
                and write->read chains must stay monotone)."""
                m = {}
                for i, fn in enumerate(cls):
                    m.setdefault(ks[i * len(ks) // len(cls)], []).append(fn)
                return m

            def merge(*maps):
                out = {}
                for m in maps:
                    for k, v in m.items():
                        out.setdefault(k, []).extend(v)
                return out

            # prologue: project s0 (K,Q first for scores; V right behind)
            for e in (1, 0, 2):
                st = {}
                for d0, d1 in ((0, 3), (3, 6), (6, 8)):
                    mk_qkv_mms(0, e, d0, d1, st)()
                if e == 2:
                    for j in range(0, 4):
                        mk_vtrans(j)()

            # iteration plan: (b, q, qkv closures -> slots, dve_ks)
            prev = None
            plans = [
                (0, 0, [(qkv_closures(1, dma_s=3), [0, 1, 2, 3]),
                        (qkv_closures(2), [4, 5, 6, 7]),
                        (qkv_closures(3, dma_s=4), [8, 9, 10, 11])], ()),
                (0, 1, [(qkv_closures(4, dma_s=5), list(range(16)))],
                 (5, 11)),
                (0, 2, [(qkv_closures(5, dma_s=6), list(range(16)))],
                 (5, 11)),
                (0, 3, [(qkv_closures(6, dma_s=7), list(range(16)))],
                 (5, 11)),
                (1, 0, [(qkv_closures(7), list(range(12)))], (5, 11)),
                (1, 1, [], (2, 6, 10, 14)),
                (1, 2, [], (2, 6, 10, 14)),
                (1, 3, [], (2, 6, 10, 14)),
            ]
            FIN_KS = [1, 2, 3, 4, 5, 6, 7, 8, 9, 10, 11, 12, 13]
            for b, q, qkvs, dve_ks in plans:
                if K_NO_I16:
                    dve_ks = ()
                if K_NOFILL:
                    # baseline-style: everything at iteration boundaries
                    if prev is not None:
                        for fn in finish_closures(prev):
                            fn()
                    for cls, ks in qkvs:
                        for fn in cls:
                            fn()
                    prev = emit_attn(b, q, {}, dve_ks)
                    continue
                kmap = merge(*[spread(cls, ks) for cls, ks in qkvs])
                if prev is not None:
                    fin = finish_closures(prev)
                    kmap = merge(kmap, {k: [fn] for k, fn in
                                        zip(FIN_KS, fin)})
                prev = emit_attn(b, q, kmap, dve_ks)
            # tail: last iteration's finish, emitted back-to-back (the
            # closures pipeline across PE/ACT/DVE/DMA on their own)
            for fn in finish_closures(prev):
                fn()

    nc.compile()
    return nc


def _get_nc():
    if "nc" not in _cache:
        _cache["nc"] = _build()
    return _cache["nc"]


def _prep_inputs(x, w_qkv, w_out):
    import ml_dtypes
    bf16 = ml_dtypes.bfloat16
    x = np.asarray(x, dtype=np.float32)
    w_qkv = np.asarray(w_qkv, dtype=np.float32)
    w_out = np.asarray(w_out, dtype=np.float32)
    xT = np.ascontiguousarray(x.reshape(BS, D).T.astype(bf16))
    in_maps = []
    for c in range(N_CORES):
        # reference splits qkv as (v, q, k): v rows [0,D), q [D,2D), k [2D,3D)
        wq = w_qkv[D + 128 * c: D + 128 * (c + 1)]
        wk = w_qkv[2 * D + 128 * c: 2 * D + 128 * (c + 1)]
        wv = w_qkv[128 * c: 128 * (c + 1)]
        wqkvT = np.ascontiguousarray(
            np.concatenate([wq, wk, wv], axis=0).T.astype(bf16))
        woutT = np.ascontiguousarray(
            w_out[:, 128 * c:128 * (c + 1)].T.astype(bf16))
        in_maps.append({"xT": xT, "wqkvT": wqkvT, "woutT": woutT})
    return in_maps


def kernel(x, w_qkv, w_out, b_out):
    from concourse.bass_utils import run_bass_kernel_spmd

    nc = _get_nc()
    in_maps = _prep_inputs(x, w_qkv, w_out)
    b_out = np.asarray(b_out, dtype=np.float32)
    res = run_bass_kernel_spmd(nc, in_maps, core_ids=list(range(N_CORES)))
    acc = np.zeros((BS, D), np.float32)
    for c in range(N_CORES):
        acc += res.results[c]["out"].astype(np.float32)
    acc = acc + b_out[None, :]
    return acc.reshape(B, S, D)
